# revision 25
# baseline (speedup 1.0000x reference)
"""Trainium2 Bass kernel for nn_DDKFLayer (windowed-FFT magnitude gating layer).

Math (derived from the reference):
  interp = cubic-polyphase upsample of signal (B,512) -> (B,2045)   [exact: t_p = p/4]
  K = g0*(interp+1.3)^2 + g1*exp(-0.5*(interp-0.7)^2),  g = softmax(gamma_logits)
  For window w (start 4w, width 20) and freq k:
    F_w[k] = e^{-i th} G_w[k],  th = 2pi*4wk/n,  G_w[k] = sum_m K[4w+m] e^{-2pi i mk/n}
    M  = |F_w| = |G_w| = sqrt(g^2 + h^2)            (g,h: 20-tap matmuls)
    M1^2 = |FFT(K) - F_w|^2 = R + P - 2c*X - 2s*Y   (R=A^2+B^2, P=g^2+h^2,
                                                     X=Ag+Bh, Y=Bg-Ah, c,s=cos/sin th)
  out = strong * sqrt(P * clip(M1^2, 0, 1)),  strong = P > beta^2 * max_k P
  Spectrum of a real signal is symmetric: compute k=0..1022, mirror 1023..2044.

Sharding: batch 32 -> 4 rows per core across 8 NeuronCores (pure data parallel).
"""
import os
import sys

os.environ.setdefault("JAX_PLATFORMS", "axon,cpu")
for _p in ("/root/.axon_site/_ro/trn_rl_repo", "/opt/trn_rl_repo"):
    if os.path.isdir(_p) and _p not in sys.path:
        sys.path.insert(0, _p)

import numpy as np

B, L = 32, 512
NCORES = 8
BPC = B // NCORES              # 4 batch rows per core
WINDOW, STEP = 20, 4
N = 2045                       # interp length
W = 507                        # number of windows
KH = 1023                      # half spectrum (k = 0..1022)
KPAD = 2068                    # K row padded so shifted window reads stay in bounds
WTILES = [(0, 128), (128, 128), (256, 128), (384, 123)]
KBLK = [(0, 512), (512, 511)]              # half-spectrum split into PSUM banks
IBLK = [(0, 512), (512, 512), (1024, 512), (1536, 509)]  # interp (2045) bank split

_STATE = {}


def _cubic_w():
    a = -0.75
    Wt = np.zeros((4, 4), np.float64)
    for r in range(4):
        f = r / 4.0
        fp1, fm1, fm2 = 1.0 + f, 1.0 - f, 2.0 - f
        Wt[r, 0] = a * fp1**3 - 5 * a * fp1**2 + 8 * a * fp1 - 4 * a
        Wt[r, 1] = (a + 2) * f**3 - (a + 3) * f**2 + 1.0
        Wt[r, 2] = (a + 2) * fm1**3 - (a + 3) * fm1**2 + 1.0
        Wt[r, 3] = a * fm2**3 - 5 * a * fm2**2 + 8 * a * fm2 - 4 * a
    return Wt


def _consts():
    if "consts" in _STATE:
        return _STATE["consts"]
    f32 = np.float32
    Wt = _cubic_w()
    # polyphase lhsT (tau, r): interp_rb[r, b*512+q] = sum_tau WP4[tau,r]*ss[tau, b*512+q]
    WP4 = np.ascontiguousarray(Wt.T)

    j = np.arange(2048)[:, None]
    k = np.arange(KH)[None, :]
    ang = 2 * np.pi * ((j * k) % N) / N
    DFTC = np.cos(ang)
    DFTS = np.sin(ang)
    DFTC[N:] = 0.0
    DFTS[N:] = 0.0

    m = np.arange(WINDOW) % WINDOW
    mb = (np.arange(4 * WINDOW) % WINDOW)[:, None]          # (80,1) tiled over b
    angm = 2 * np.pi * ((mb * k) % N) / N
    C80 = np.cos(angm)
    S80 = np.sin(angm)

    # rotation tables packed as (128, 4*1023): column block wt holds rows w=wt*128+p
    wfull = np.arange(512)[:, None]                          # padded to 512
    angw = 2 * np.pi * ((STEP * wfull * k) % N) / N
    c2 = 2 * np.cos(angw)
    s2 = 2 * np.sin(angw)
    c2[W:] = 0.0
    s2[W:] = 0.0
    C2P = c2.reshape(4, 128, KH).transpose(1, 0, 2).reshape(128, 4 * KH)
    S2P = s2.reshape(4, 128, KH).transpose(1, 0, 2).reshape(128, 4 * KH)

    SEL80 = np.zeros((BPC, 80), np.float64)
    for b in range(BPC):
        SEL80[b, b * 20:(b + 1) * 20] = 1.0
    SELRB = np.zeros((BPC, 512), np.float64)
    for b in range(BPC):
        SELRB[b, b * 128:(b + 1) * 128] = 1.0

    import ml_dtypes
    bf16 = ml_dtypes.bfloat16
    _STATE["consts"] = {
        "wp4": WP4.astype(f32), "dftc": DFTC.astype(f32), "dfts": DFTS.astype(f32),
        "c80": C80.astype(f32), "s80": S80.astype(f32),
        "c2p": C2P.astype(f32), "s2p": S2P.astype(f32),
        "sel80": SEL80.astype(f32), "selrb": SELRB.astype(f32),
    }
    return _STATE["consts"]


def _build():
    if "nc" in _STATE:
        return _STATE["nc"]
    import concourse.bass as bass
    import concourse.bacc as bacc
    import concourse.mybir as mybir
    import concourse.tile as tile

    F32 = mybir.dt.float32
    AF = mybir.ActivationFunctionType
    OP = mybir.AluOpType
    AX = mybir.AxisListType

    nc = bacc.Bacc("TRN2", target_bir_lowering=False, debug=False, num_devices=NCORES)
    BF16 = mybir.dt.bfloat16
    rowst = lambda t: t[:].ap[0][0]   # true partition stride (elements)

    ss_d = nc.declare_dram_parameter("ss", [4, 4 * L], F32, isOutput=False)
    beta_d = nc.declare_dram_parameter("beta", [1, 1], F32, isOutput=False)
    gl_d = nc.declare_dram_parameter("gl", [1, 2], F32, isOutput=False)
    wp4_d = nc.declare_dram_parameter("wp4", [4, 4], F32, isOutput=False)
    dftc_d = nc.declare_dram_parameter("dftc", [2048, KH], F32, isOutput=False)
    dfts_d = nc.declare_dram_parameter("dfts", [2048, KH], F32, isOutput=False)
    c80_d = nc.declare_dram_parameter("c80", [80, KH], F32, isOutput=False)
    s80_d = nc.declare_dram_parameter("s80", [80, KH], F32, isOutput=False)
    c2p_d = nc.declare_dram_parameter("c2p", [128, 4 * KH], F32, isOutput=False)
    s2p_d = nc.declare_dram_parameter("s2p", [128, 4 * KH], F32, isOutput=False)
    sel80_d = nc.declare_dram_parameter("sel80", [BPC, 80], F32, isOutput=False)
    selrb_d = nc.declare_dram_parameter("selrb", [BPC, 512], F32, isOutput=False)
    out_d = nc.declare_dram_parameter("out", [BPC, W, N], F32, isOutput=True)
    # internal DRAM scratch for cross-partition rearranges
    scrb_d = nc.dram_tensor("scrb", [4, 4 * L], F32)    # K in (r, b*512+q) layout
    scrk_d = nc.dram_tensor("scrk", [BPC, KPAD], F32)   # K in (b, j) layout

    with tile.TileContext(nc) as tc:
        with tc.tile_pool(name="cst", bufs=1) as cst:
            # ---- resident constants ----
            c80_sb = cst.tile([80, KH], F32)
            nc.sync.dma_start(c80_sb[:], c80_d[:])
            s80_sb = cst.tile([80, KH], F32)
            nc.sync.dma_start(s80_sb[:], s80_d[:])
            c2_sb = cst.tile([128, 4 * KH], F32)
            nc.sync.dma_start(c2_sb[:], c2p_d[:])
            s2_sb = cst.tile([128, 4 * KH], F32)
            nc.sync.dma_start(s2_sb[:], s2p_d[:])
            sel80_sb = cst.tile([BPC, 80], F32)
            nc.sync.dma_start(sel80_sb[:], sel80_d[:])
            selrb_sb = cst.tile([BPC, 512], F32)
            nc.sync.dma_start(selrb_sb[:], selrb_d[:])
            ss_sb = cst.tile([4, 4 * L], F32)
            nc.sync.dma_start(ss_sb[:], ss_d[:])
            wp4_sb = cst.tile([4, 4], F32)
            nc.sync.dma_start(wp4_sb[:], wp4_d[:])
            beta_sb = cst.tile([1, 1], F32)
            nc.sync.dma_start(beta_sb[:], beta_d[:])
            gl_sb = cst.tile([1, 2], F32)
            nc.sync.dma_start(gl_sb[:], gl_d[:])
            ones14 = cst.tile([1, BPC], F32)
            nc.vector.memset(ones14[:], 1.0)
            ones128 = cst.tile([1, 128], F32)
            nc.vector.memset(ones128[:], 1.0)
            bm07 = cst.tile([BPC, 1], F32)
            nc.vector.memset(bm07[:], -0.7)
            b13 = cst.tile([BPC, 1], F32)
            nc.vector.memset(b13[:], 1.3)

            # resident per-b derived tensors
            K4 = cst.tile([BPC, KPAD], F32)
            wx_b = [cst.tile([20, KH], F32, name=f"wx{b}", tag=f"wx{b}")
                    for b in range(BPC)]
            wy_b = [cst.tile([20, KH], F32, name=f"wy{b}", tag=f"wy{b}")
                    for b in range(BPC)]
            rbc_b = [cst.tile([128, KH], F32, name=f"rbc{b}", tag=f"rbc{b}")
                     for b in range(BPC)]
            ktall = cst.tile([128, 4 * 16], F32)
            b2bc = cst.tile([128, 1], F32)
            gb = cst.tile([BPC, 2], F32)

            # ================= setup =================
            with (
                tc.tile_pool(name="stp_sb", bufs=1) as ssb,
                tc.tile_pool(name="stp_ps", bufs=2, space=bass.MemorySpace.PSUM) as sps,
            ):
                # ---- interp via polyphase matmul: (4r, b*512+q) layout ----
                psI = sps.tile([4, 4 * L], F32, tag="sp")
                for blk in range(4):
                    nc.tensor.matmul(
                        psI[:, blk * 512:(blk + 1) * 512],
                        wp4_sb[:],
                        ss_sb[:, blk * 512:(blk + 1) * 512],
                        start=True, stop=True)

                # ---- K (ACT: Square in sqrt set, Exp in exp set) ----
                t07 = ssb.tile([4, 4 * L], F32, tag="kp1")
                nc.scalar.activation(t07[:], psI[:], AF.Square, bias=bm07[:])
                poly = ssb.tile([4, 4 * L], F32, tag="kp2")
                nc.scalar.activation(poly[:], psI[:], AF.Square, bias=b13[:])
                gauss = ssb.tile([4, 4 * L], F32, tag="kp3")
                nc.scalar.activation(gauss[:], t07[:], AF.Exp, scale=-0.5)
                ge = ssb.tile([1, 2], F32, tag="ge")
                nc.scalar.activation(ge[:], gl_sb[:], AF.Exp)

                # gamma = softmax(gl); broadcast to (BPC,1) scalars
                gs = ssb.tile([1, 1], F32, tag="gs")
                nc.vector.tensor_reduce(gs[:], ge[:], axis=AX.X, op=OP.add)
                gr = ssb.tile([1, 1], F32, tag="gr")
                nc.vector.reciprocal(gr[:], gs[:])
                gam = ssb.tile([1, 2], F32, tag="gam")
                nc.vector.tensor_scalar(gam[:], ge[:], gr[:, 0:1], None, op0=OP.mult)
                psg = sps.tile([BPC, 2], F32, tag="sp")
                nc.tensor.matmul(psg[:], ones14[:], gam[:], start=True, stop=True)
                nc.scalar.copy(gb[:], psg[:])

                # beta^2 broadcast to (128,1)
                bsq = ssb.tile([1, 1], F32, tag="bsq")
                nc.scalar.activation(bsq[:], beta_sb[:], AF.Square)
                psb2 = sps.tile([128, 1], F32, tag="sp")
                nc.tensor.matmul(psb2[:], ones128[:], bsq[:], start=True, stop=True)
                nc.scalar.copy(b2bc[:], psb2[:])

                # K = g0*poly + g1*gauss (still in (r, b*512+q) layout)
                pre = ssb.tile([4, 4 * L], F32, tag="kp1")
                nc.vector.tensor_scalar(pre[:], gauss[:], gb[:, 1:2], None, op0=OP.mult)
                krb = cst.tile([4, 4 * L], F32, name="krb")  # resident: window source
                nc.vector.scalar_tensor_tensor(
                    krb[:], poly[:], gb[:, 0:1], pre[:], op0=OP.mult, op1=OP.add)

                # ---- rearrange K to (b, j) layout via DRAM bounce ----
                # K4[b, 4q+r] = krb[r, b*512+q]
                nc.vector.memset(K4[:], 0.0)
                nc.sync.dma_start(scrb_d[:], krb[:])
                k4st = rowst(K4)
                for r in range(4):
                    cnt = 512 if r == 0 else 511
                    nc.sync.dma_start(
                        bass.AP(K4[:].tensor, K4[:].offset + r,
                                [[k4st, BPC], [STEP, cnt]]),
                        bass.AP(scrb_d[:].tensor, r * 4 * L, [[L, BPC], [1, cnt]]))

                # ---- K^T chunks via DRAM bounce: ktall[j, c*4+b] = K[b, c*128+j] ----
                nc.sync.dma_start(scrk_d[:], K4[:])
                ktst = rowst(ktall)
                for b in range(BPC):
                    nc.sync.dma_start(
                        bass.AP(ktall[:].tensor, ktall[:].offset + b,
                                [[ktst, 128], [BPC, 16]]),
                        bass.AP(scrk_d[:].tensor, b * KPAD, [[1, 128], [128, 16]]))

                # ---- A, B (full DFT of K on half spectrum) ----
                psA = sps.tile([BPC, KH], F32, tag="sp")
                psB = sps.tile([BPC, KH], F32, tag="sp")
                for c in range(16):
                    dc = ssb.tile([128, KH], F32, tag="dc", bufs=2)
                    nc.sync.dma_start(dc[:], dftc_d[c * 128:(c + 1) * 128, :])
                    ds = ssb.tile([128, KH], F32, tag="ds", bufs=2)
                    nc.sync.dma_start(ds[:], dfts_d[c * 128:(c + 1) * 128, :])
                    for (k0, kn) in KBLK:
                        nc.tensor.matmul(psA[:, k0:k0 + kn],
                                         ktall[:, c * BPC:(c + 1) * BPC],
                                         dc[:, k0:k0 + kn],
                                         start=(c == 0), stop=(c == 15))
                        nc.tensor.matmul(psB[:, k0:k0 + kn],
                                         ktall[:, c * BPC:(c + 1) * BPC],
                                         ds[:, k0:k0 + kn],
                                         start=(c == 0), stop=(c == 15))

                A_sb = ssb.tile([BPC, KH], F32, tag="A_sb")
                nc.scalar.copy(A_sb[:], psA[:])
                B_sb = ssb.tile([BPC, KH], F32, tag="B_sb")
                nc.scalar.copy(B_sb[:], psB[:])
                Asq = ssb.tile([BPC, KH], F32, tag="Asq")
                nc.scalar.activation(Asq[:], psA[:], AF.Square)
                Bsq = ssb.tile([BPC, KH], F32, tag="Bsq")
                nc.scalar.activation(Bsq[:], psB[:], AF.Square)
                R4 = ssb.tile([BPC, KH], F32, tag="R4")
                nc.vector.tensor_add(R4[:], Asq[:], Bsq[:])

                # ---- W_X / W_Y rhs tables: (80,1023) batch then scatter per b ----
                psA80 = sps.tile([80, KH], F32, tag="sp")
                psB80 = sps.tile([80, KH], F32, tag="sp")
                for (k0, kn) in KBLK:
                    nc.tensor.matmul(psA80[:, k0:k0 + kn], sel80_sb[:],
                                     A_sb[:, k0:k0 + kn], start=True, stop=True)
                    nc.tensor.matmul(psB80[:, k0:k0 + kn], sel80_sb[:],
                                     B_sb[:, k0:k0 + kn], start=True, stop=True)
                tAC = ssb.tile([80, KH], F32, tag="tAC")
                nc.vector.tensor_mul(tAC[:], c80_sb[:], psA80[:])
                tBS = ssb.tile([80, KH], F32, tag="tBS")
                nc.vector.tensor_mul(tBS[:], s80_sb[:], psB80[:])
                wx80 = ssb.tile([80, KH], F32, tag="wx80")
                nc.vector.tensor_add(wx80[:], tAC[:], tBS[:])
                tBC = ssb.tile([80, KH], F32, tag="tBC")
                nc.vector.tensor_mul(tBC[:], c80_sb[:], psB80[:])
                tAS = ssb.tile([80, KH], F32, tag="tAS")
                nc.vector.tensor_mul(tAS[:], s80_sb[:], psA80[:])
                wy80 = ssb.tile([80, KH], F32, tag="wy80")
                nc.vector.tensor_sub(wy80[:], tBC[:], tAS[:])
                wst = rowst(wx80)
                for b in range(BPC):
                    nc.sync.dma_start(
                        wx_b[b][:],
                        bass.AP(wx80[:].tensor, wx80[:].offset + b * 20 * wst,
                                [[wst, 20], [1, KH]]))
                    nc.sync.dma_start(
                        wy_b[b][:],
                        bass.AP(wy80[:].tensor, wy80[:].offset + b * 20 * wst,
                                [[wst, 20], [1, KH]]))

                # ---- R broadcast per b: rbc_b[p,k] = R[b,k] ----
                for b in range(BPC):
                    psR = sps.tile([128, KH], F32, tag="sp")
                    for (k0, kn) in KBLK:
                        nc.tensor.matmul(psR[:, k0:k0 + kn],
                                         selrb_sb[:, b * 128:(b + 1) * 128],
                                         R4[:, k0:k0 + kn], start=True, stop=True)
                    nc.scalar.copy(rbc_b[b][:], psR[:])

            # ================= main loop =================
            with (
                tc.tile_pool(name="mwk", bufs=2) as wk,
                tc.tile_pool(name="mout", bufs=2) as owk,
                tc.tile_pool(name="mps", bufs=2, space=bass.MemorySpace.PSUM) as mps,
            ):
                for b in range(BPC):
                    for (w0, P) in WTILES:
                        wt = w0 // 128
                        # kwin[4h+r, wi] = K[b, 4*(w0+wi)+4h+r] = krb[r, b*512+w0+wi+h]
                        kwin = wk.tile([20, 128], F32, tag="kwin")
                        kst = rowst(kwin)
                        krst = rowst(krb)
                        for r in range(4):
                            nc.sync.dma_start(
                                bass.AP(kwin[:].tensor, kwin[:].offset + r * kst,
                                        [[4 * kst, 5], [1, P]]),
                                bass.AP(krb[:].tensor,
                                        krb[:].offset + r * krst + b * L + w0,
                                        [[krst, 1], [1, 5], [1, P]]))

                        pw = wk.tile([128, KH], F32, tag="pw")
                        sq = wk.tile([128, KH], F32, tag="sq")
                        reds = wk.tile([128, 2], F32, tag="reds")
                        ost = owk.tile([128, N], F32, tag="ost")

                        for kbi, (k0, kn) in enumerate(KBLK):
                            psG = mps.tile([128, 512], F32, tag="psG")
                            psH = mps.tile([128, 512], F32, tag="psH")
                            psX = mps.tile([128, 512], F32, tag="psX")
                            psY = mps.tile([128, 512], F32, tag="psY")
                            nc.tensor.matmul(psG[:P, :kn], kwin[:, :P],
                                             c80_sb[0:20, k0:k0 + kn],
                                             start=True, stop=True)
                            nc.tensor.matmul(psH[:P, :kn], kwin[:, :P],
                                             s80_sb[0:20, k0:k0 + kn],
                                             start=True, stop=True)
                            nc.tensor.matmul(psX[:P, :kn], kwin[:, :P],
                                             wx_b[b][:, k0:k0 + kn],
                                             start=True, stop=True)
                            nc.tensor.matmul(psY[:P, :kn], kwin[:, :P],
                                             wy_b[b][:, k0:k0 + kn],
                                             start=True, stop=True)

                            gsq = wk.tile([128, 512], F32, tag="gsq")
                            nc.scalar.activation(gsq[:P, :kn], psG[:P, :kn], AF.Square)
                            hsq = wk.tile([128, 512], F32, tag="hsq")
                            nc.scalar.activation(hsq[:P, :kn], psH[:P, :kn], AF.Square)
                            nc.vector.tensor_add(pw[:P, k0:k0 + kn],
                                                 gsq[:P, :kn], hsq[:P, :kn])
                            nc.vector.tensor_reduce(reds[:P, kbi:kbi + 1],
                                                    pw[:P, k0:k0 + kn],
                                                    axis=AX.X, op=OP.max)

                            t1 = wk.tile([128, 512], F32, tag="t1")
                            nc.vector.tensor_mul(
                                t1[:P, :kn],
                                c2_sb[:P, wt * KH + k0: wt * KH + k0 + kn],
                                psX[:P, :kn])
                            t2 = wk.tile([128, 512], F32, tag="t2")
                            nc.vector.tensor_mul(
                                t2[:P, :kn],
                                s2_sb[:P, wt * KH + k0: wt * KH + k0 + kn],
                                psY[:P, :kn])
                            t12 = wk.tile([128, 512], F32, tag="t12")
                            nc.vector.tensor_add(t12[:P, :kn], t1[:P, :kn], t2[:P, :kn])
                            pr = wk.tile([128, 512], F32, tag="pr")
                            nc.vector.tensor_add(pr[:P, :kn], pw[:P, k0:k0 + kn],
                                                 rbc_b[b][:P, k0:k0 + kn])
                            qv = wk.tile([128, 512], F32, tag="qv")
                            nc.vector.tensor_sub(qv[:P, :kn], pr[:P, :kn], t12[:P, :kn])
                            # qm = clip(qv, 0, 1); zm = qm * pw; sq = sqrt(zm)
                            qm = wk.tile([128, 512], F32, tag="qm")
                            nc.vector.tensor_scalar(qm[:P, :kn], qv[:P, :kn],
                                                    1.0, 0.0, op0=OP.min, op1=OP.max)
                            zm = wk.tile([128, 512], F32, tag="zm")
                            nc.vector.tensor_mul(zm[:P, :kn], qm[:P, :kn],
                                                 pw[:P, k0:k0 + kn])
                            nc.scalar.activation(sq[:P, k0:k0 + kn], zm[:P, :kn],
                                                 AF.Sqrt)

                        thr = wk.tile([128, 1], F32, tag="thr")
                        nc.vector.tensor_tensor(thr[:P], reds[:P, 0:1],
                                                reds[:P, 1:2], op=OP.max)
                        nc.vector.tensor_mul(thr[:P], thr[:P], b2bc[:P])
                        for (k0, kn) in KBLK:
                            nc.vector.scalar_tensor_tensor(
                                ost[:P, k0:k0 + kn], pw[:P, k0:k0 + kn], thr[:P],
                                sq[:P, k0:k0 + kn], op0=OP.is_gt, op1=OP.mult)
                        nc.vector.tensor_copy(ost[:P, KH:N],
                                              ost[:P, 1:KH][:, ::-1])
                        nc.sync.dma_start(out_d[b, w0:w0 + P, :], ost[:P, :])

    nc.compile()
    _STATE["nc"] = nc
    return nc


def _ensure_ntff_hook():
    """Shim antenv.axon_hooks (absent in this image) so trace=True works."""
    import types

    try:
        from antenv.axon_hooks import get_axon_ntff_profile_hook  # noqa: F401
        return
    except ImportError:
        pass
    mod = types.ModuleType("antenv.axon_hooks")
    _h = {"hook": None}
    mod.set_axon_ntff_profile_hook = lambda h: _h.__setitem__("hook", h)
    mod.get_axon_ntff_profile_hook = lambda: _h["hook"]
    import antenv
    antenv.axon_hooks = mod
    sys.modules["antenv.axon_hooks"] = mod
    try:
        from trn_agent_boot.trn_boot import _ntff_profile_via_ctypes
        mod.set_axon_ntff_profile_hook(
            _ntff_profile_via_ctypes("/opt/axon/libaxon_pjrt.so"))
    except Exception as e:  # pragma: no cover
        print(f"ntff hook setup failed: {e}", file=sys.stderr)


def _run(inputs, trace=False):
    from concourse.bass_utils import run_bass_kernel_spmd

    if trace:
        _ensure_ntff_hook()

    nc = _build()
    consts = _consts()
    signal = np.ascontiguousarray(np.asarray(inputs["signal"], np.float32))
    beta = np.asarray(inputs["beta"], np.float32).reshape(1, 1)
    gl = np.asarray(inputs["gamma_logits"], np.float32).reshape(1, 2)

    # sigshift[tau, b*512+q] = sh[b, clamp(q-1+tau, 0, 511)]
    qv = np.arange(L)
    idx = np.clip(qv[None, :] - 1 + np.arange(4)[:, None], 0, L - 1)  # (4, 512)
    in_maps = []
    for core in range(NCORES):
        sh = signal[core * BPC:(core + 1) * BPC]          # (4, 512)
        ss = np.ascontiguousarray(
            sh[:, idx].transpose(1, 0, 2).reshape(4, BPC * L))  # (tau, b*512+q)
        in_maps.append({
            "ss": ss, "beta": beta, "gl": gl, "wp4": consts["wp4"],
            "dftc": consts["dftc"], "dfts": consts["dfts"],
            "c80": consts["c80"], "s80": consts["s80"],
            "c2p": consts["c2p"], "s2p": consts["s2p"],
            "sel80": consts["sel80"], "selrb": consts["selrb"],
        })
    res = run_bass_kernel_spmd(nc, in_maps, list(range(NCORES)), trace=trace)
    out = np.concatenate([res.results[c]["out"] for c in range(NCORES)], axis=0)
    return out, res


def kernel(signal, alpha=None, beta=None, gamma_logits=None, **_):
    out, _res = _run({"signal": signal, "beta": beta, "gamma_logits": gamma_logits})
    return out
